# revision 1
# baseline (speedup 1.0000x reference)
"""AdapLSNet MLP kernel for 8 TRN2 NeuronCores (data-parallel).

reference:
    h  = elu(x @ W0 + b0)
    h  = elu(h @ W1 + b1)
    out = sigmoid(h @ W2 + b2)          # [B, 1]
    alpha = piecewise(out)               # a=0.1, b=0.2, c=0.8
    returns (out, alpha)

Strategy
- Shard batch (32768) across 8 cores (4096 rows each); replicate weights.
- Host pre-transposes each x shard to x^T [1024, 4096] so every layer's
  activations live in [feature(partitions), batch(free)] layout; weights
  W0/W1 are already in [K, M] layout for the stationary operand.  No
  on-device transposes.
- All three layers in fp16 (full PE rate, FWL weight loads, half the
  DMA/SBUF bytes; measured end-to-end rel err 1.3e-3 vs the 2e-2 gate).
- Single fused pass: W0 (fp16, 32KB/part) and W1 (fp16, 64KB/part) are
  both SBUF-resident, so h1 never leaves the chip.  Weight slabs stay
  [128, 2048] (LDWEIGHTS from small tiles measures 1.2-2.5x slower);
  their DMAs are split into strips across the sync (HWDGE) and gpsimd
  (SWDGE) queue families to roughly double preload bandwidth, ordered
  first-use-first so L1 starts as soon as xt0 + the first slabs land.
- Software pipeline: L1 runs one batch-chunk ahead of L2
  (L1_0, L1_1, L2_0, L1_2, L2_1, ... L2_7) so the PE has L1 work while
  W1 streams in at startup.
- L3 (h2 @ W2, M=1) is packed 4-wide into PE column groups via
  tile_position so 4 accumulation matmuls run concurrently; the partial
  rows on psum partitions 0/32/64/96 are reduced on ScalarE/VectorE.
- elu(z) = min(exp(z) - 1, relu(z)): 2 ScalarE LUT ops reading PSUM with
  the bias fused + 1 fused VectorE (e-1) min r op.
- alpha = relu(-0.5*out + 0.1) + relu(0.5*out - 0.4)  (exact identity for
  out in [0,1]).
"""

import numpy as np

BATCH = 32768
DIN = 1024
DH = 2048
NCORES = 8
SHARD = BATCH // NCORES          # 4096
CHUNK = 512
NCH = SHARD // CHUNK             # 8
KI = DIN // 128                  # 8
KH = DH // 128                   # 16
MH = DH // 128                   # 16
NH1S = 48                        # h1 slots (fp16 [128,512], 1KB each; 3 chunks)
NXTS = 14                        # xt slots (f32r [128,512], 2KB each)


def _install_profile_shim():
    """Allow trace=True under axon (exec_time_ns capture) if possible."""
    import sys
    import types

    try:
        import antenv

        if "antenv.axon_hooks" in sys.modules:
            return
        mod = types.ModuleType("antenv.axon_hooks")
        _hook = [None]
        mod.set_axon_ntff_profile_hook = lambda h: _hook.__setitem__(0, h)
        mod.get_axon_ntff_profile_hook = lambda: _hook[0]
        sys.modules["antenv.axon_hooks"] = mod
        antenv.axon_hooks = mod
        try:
            from trn_agent_boot.trn_boot import _ntff_profile_via_ctypes

            mod.set_axon_ntff_profile_hook(
                _ntff_profile_via_ctypes("/opt/axon/libaxon_pjrt.so")
            )
        except Exception:
            pass
    except Exception:
        pass


_NC_CACHE = None


def _build():
    global _NC_CACHE
    if _NC_CACHE is not None:
        return _NC_CACHE

    import concourse.mybir as mybir
    import concourse.tile as tile
    from concourse import bacc

    F32R = mybir.dt.float32r
    F32 = mybir.dt.float32
    F16 = mybir.dt.float16
    AF = mybir.ActivationFunctionType
    ALU = mybir.AluOpType

    nc = bacc.Bacc("TRN2", target_bir_lowering=False)

    xt_ext = nc.declare_dram_parameter("xt", [DIN, SHARD], F16, isOutput=False)
    w0_ext = nc.declare_dram_parameter("w0", [DIN, DH], F16, isOutput=False)
    w1_ext = nc.declare_dram_parameter("w1", [DH, DH], F16, isOutput=False)
    w2_ext = nc.declare_dram_parameter("w2", [128, KH], F16, isOutput=False)
    b0_ext = nc.declare_dram_parameter("b0", [128, MH], F32, isOutput=False)
    b1_ext = nc.declare_dram_parameter("b1", [128, MH], F32, isOutput=False)
    b2_ext = nc.declare_dram_parameter("b2", [1, 1], F32, isOutput=False)
    out_ext = nc.declare_dram_parameter("out", [1, SHARD], F32, isOutput=True)
    alpha_ext = nc.declare_dram_parameter("alpha", [1, SHARD], F32, isOutput=True)

    with tile.TileContext(nc) as tc:
        with (
            tc.tile_pool(name="w0p", bufs=1) as w0p,
            tc.tile_pool(name="w1p", bufs=1) as w1p,
            tc.tile_pool(name="xtp", bufs=1) as xtp,
            tc.tile_pool(name="h1p", bufs=1) as h1p,
            tc.tile_pool(name="hpool", bufs=2) as hpool,
            tc.tile_pool(name="h2p", bufs=5) as h2p,
            tc.tile_pool(name="redp", bufs=3) as redp,
            tc.tile_pool(name="cst", bufs=1) as cst,
            tc.tile_pool(name="ps", bufs=6, space="PSUM") as ps,
            tc.tile_pool(name="ops", bufs=2, space="PSUM") as ops,
        ):
            # weights live in full [128, DH] slabs: LDWEIGHTS from small
            # tiles measures 1.2-2.5x slower, so slabs stay big and DMA
            # completion is slab-granular
            w0_sb = [
                w0p.tile([128, DH], F16, tag=f"w0_{k}", name=f"w0_{k}")
                for k in range(KI)
            ]
            w1_sb = [
                w1p.tile([128, DH], F16, tag=f"w1_{k}", name=f"w1_{k}")
                for k in range(KH)
            ]

            def w0_lhsT(k, m):
                return w0_sb[k][:, m * 128:(m + 1) * 128]

            def w1_lhsT(k, m):
                return w1_sb[k][:, m * 128:(m + 1) * 128]

            def emit_xt(n, halves=False):
                tiles = []
                base = (KI * n) % NXTS
                for k in range(KI):
                    t = xtp.tile(
                        [128, CHUNK], F16, tag=f"xt{(base + k) % NXTS}",
                        name=f"xt_{n}_{k}",
                    )
                    src = xt_ext[k * 128:(k + 1) * 128,
                                 n * CHUNK:(n + 1) * CHUNK]
                    if halves:
                        nc.sync.dma_start(t[:, 0:256], src[:, 0:256])
                        nc.sync.dma_start(t[:, 256:512], src[:, 256:512])
                    else:
                        nc.sync.dma_start(t[:], src)
                    tiles.append(t)
                return tiles

            # weight slab DMAs: 4 strips per slab, alternating the sync
            # (HWDGE) and gpsimd (SWDGE) queue families to double the
            # prefetch bandwidth; slab-major so early slabs finish first
            def emit_w(sb_tiles, ext, nk, elem, nstrip, ks, mid=None,
                       sync_only_ks=()):
                for k in ks:
                    for s in range(nstrip):
                        eng = (nc.sync if (s % 2 == 0 or k in sync_only_ks)
                               else nc.gpsimd)
                        eng.dma_start(
                            sb_tiles[k][:, s * elem:(s + 1) * elem],
                            ext[k * 128:(k + 1) * 128, s * elem:(s + 1) * elem],
                        )
                    if mid is not None and k == mid[0]:
                        mid[1]()

            # startup-ordered DMA stream (first-use first).  xt1 is
            # injected between W0 slabs 2 and 3 so L1(1) is never starved
            # (SWDGE/gpsimd carries only latency-tolerant weight strips).
            xt_tiles = {0: emit_xt(0, halves=True)}
            emit_w(
                w0_sb, w0_ext, KI, DH // 8, 8, range(KI),
                mid=(2, lambda: xt_tiles.__setitem__(1, emit_xt(1, halves=True))),
            )
            xt_tiles[2] = emit_xt(2, halves=True)
            emit_w(w1_sb, w1_ext, KH, DH // 4, 4, range(KH))

            w2_sb = cst.tile([128, KH], F16, tag="w2", name="w2")
            nc.sync.dma_start(w2_sb[:], w2_ext[:])
            b0_sb = cst.tile([128, MH], F32, tag="b0", name="b0")
            nc.sync.dma_start(b0_sb[:], b0_ext[:])
            b1_sb = cst.tile([128, MH], F32, tag="b1", name="b1")
            nc.sync.dma_start(b1_sb[:], b1_ext[:])
            b2_sb = cst.tile([1, 1], F32, tag="b2", name="b2")
            nc.sync.dma_start(b2_sb[:], b2_ext[:])
            c_b1 = cst.tile([1, 1], F32, tag="c_b1", name="c_b1")
            c_b2 = cst.tile([1, 1], F32, tag="c_b2", name="c_b2")
            c_sn = cst.tile([1, 1], F32, tag="c_sn", name="c_sn")
            c_sp = cst.tile([1, 1], F32, tag="c_sp", name="c_sp")
            nc.vector.memset(c_b1[:], 0.1)
            nc.vector.memset(c_b2[:], -0.4)
            nc.vector.memset(c_sn[:], -0.5)
            nc.vector.memset(c_sp[:], 0.5)

            # PE warmup: ~48 dependency-free matmuls on a memset tile keep
            # the PE busy during the initial weight/x DMA wait so the HAM
            # clock gate is already released (2.4 GHz) when real matmuls
            # start.  Runs entirely inside otherwise-idle time.
            wu = hpool.tile([128, CHUNK], F16, tag="e", name="wu")
            nc.vector.memset(wu[:], 0.0)
            for i in range(48):
                wps = ops.tile([128, CHUNK], F32, tag="ops", name=f"wups_{i}")
                nc.tensor.matmul(
                    wps[:], wu[:, 0:128], wu[:], start=True, stop=True,
                )

            h1_tiles = {}

            def l1_chunk(n):
                """L1: h1(n) = elu(W0.T @ xT(n) + b0), kept in SBUF."""
                xt_sb = xt_tiles.pop(n)
                h1base = (MH * n) % NH1S
                tiles = []
                for m in range(MH):
                    psum = ps.tile([128, CHUNK], F32, tag="ps",
                                   name=f"psA_{n}_{m}")
                    for k in range(KI):
                        nc.tensor.matmul(
                            psum[:], w0_lhsT(k, m), xt_sb[k][:],
                            start=(k == 0), stop=(k == KI - 1),
                        )
                    e = hpool.tile([128, CHUNK], F32, tag="e", name="e")
                    r = hpool.tile([128, CHUNK], F32, tag="r", name="r")
                    nc.scalar.activation(e[:], psum[:], AF.Exp,
                                         bias=b0_sb[:, m:m + 1])
                    nc.scalar.activation(r[:], psum[:], AF.Relu,
                                         bias=b0_sb[:, m:m + 1])
                    h1 = h1p.tile(
                        [128, CHUNK], F16, tag=f"h{(h1base + m) % NH1S}",
                        name=f"h1_{n}_{m}",
                    )
                    nc.vector.scalar_tensor_tensor(
                        h1[:], e[:], 1.0, r[:], ALU.subtract, ALU.min
                    )
                    tiles.append(h1)
                h1_tiles[n] = tiles

            def l2_chunk(n):
                """L2 + L3 + sigmoid + alpha for chunk n.

                L3 (h2 @ W2, M=1) is packed 4-wide into PE column groups
                via tile_position, issued as bursts of 4 concurrent MMs;
                the 4 partial rows (psum partitions 0/32/64/96) are
                reduced on ScalarE/VectorE.
                """
                h1_sb = h1_tiles.pop(n)
                out_ps = ops.tile([128, CHUNK], F32, tag="ops",
                                  name=f"outps_{n}")
                h2_burst = []
                for m in range(MH):
                    psum = ps.tile([128, CHUNK], F32, tag="ps",
                                   name=f"psB_{n}_{m}")
                    for k in range(KH):
                        nc.tensor.matmul(
                            psum[:], w1_lhsT(k, m), h1_sb[k][:],
                            start=(k == 0), stop=(k == KH - 1),
                        )
                    e = hpool.tile([128, CHUNK], F32, tag="e", name="e")
                    r = hpool.tile([128, CHUNK], F32, tag="r", name="r")
                    nc.scalar.activation(e[:], psum[:], AF.Exp,
                                         bias=b1_sb[:, m:m + 1])
                    nc.scalar.activation(r[:], psum[:], AF.Relu,
                                         bias=b1_sb[:, m:m + 1])
                    h2 = h2p.tile([128, CHUNK], F16, tag="h2", name="h2")
                    nc.vector.scalar_tensor_tensor(
                        h2[:], e[:], 1.0, r[:], ALU.subtract, ALU.min
                    )
                    h2_burst.append((m, h2))
                    if len(h2_burst) == 4:
                        for mm, hh in h2_burst:
                            g = mm % 4
                            nc.tensor.matmul(
                                out_ps[32 * g:32 * g + 1, :],
                                w2_sb[:, mm:mm + 1], hh[:],
                                start=(mm < 4), stop=(mm >= MH - 4),
                                tile_position=(0, 32 * g),
                            )
                        h2_burst = []
                # reduce 4 partial rows -> z, then sigmoid + alpha
                t0 = redp.tile([1, CHUNK], F32, tag="tred", name="t0")
                nc.scalar.activation(t0[:], out_ps[0:1, :], AF.Copy)
                t1 = redp.tile([1, CHUNK], F32, tag="tred", name="t1")
                nc.vector.tensor_tensor(t1[:], t0[:], out_ps[32:33, :], ALU.add)
                t2 = redp.tile([1, CHUNK], F32, tag="tred", name="t2")
                nc.vector.tensor_tensor(t2[:], t1[:], out_ps[64:65, :], ALU.add)
                z = redp.tile([1, CHUNK], F32, tag="tred", name="z")
                nc.vector.tensor_tensor(z[:], t2[:], out_ps[96:97, :], ALU.add)
                o = hpool.tile([1, CHUNK], F32, tag="e", name="o")
                nc.scalar.activation(o[:], z[:], AF.Sigmoid, bias=b2_sb[:])
                r1 = hpool.tile([1, CHUNK], F32, tag="r", name="r1")
                r2 = redp.tile([1, CHUNK], F32, tag="tred", name="r2")
                nc.scalar.activation(r1[:], o[:], AF.Relu,
                                     bias=c_b1[:], scale=c_sn[:])
                nc.scalar.activation(r2[:], o[:], AF.Relu,
                                     bias=c_b2[:], scale=c_sp[:])
                al = hpool.tile([1, CHUNK], F32, tag="e", name="al")
                nc.vector.tensor_tensor(al[:], r1[:], r2[:], ALU.add)
                nc.sync.dma_start(out_ext[0:1, n * CHUNK:(n + 1) * CHUNK], o[:])
                nc.sync.dma_start(
                    alpha_ext[0:1, n * CHUNK:(n + 1) * CHUNK], al[:]
                )

            # pipeline: L1 three chunks ahead of L2 (consume chunk n-3
            # BEFORE L1(n) writes into its ring slots - else deadlock)
            l1_chunk(0)
            l1_chunk(1)
            xt_tiles[3] = emit_xt(3)
            l1_chunk(2)
            for n in range(3, NCH):
                l2_chunk(n - 3)
                l1_chunk(n)
                if n + 1 < NCH:
                    xt_tiles[n + 1] = emit_xt(n + 1)
            for n in range(NCH - 3, NCH):
                l2_chunk(n)

    nc.compile()
    _NC_CACHE = nc
    return nc


LAST_RESULTS = None


def kernel(x, W0, b0, W1, b1, W2, b2):
    global LAST_RESULTS
    _install_profile_shim()
    from concourse.bass_utils import run_bass_kernel_spmd

    x = np.asarray(x, dtype=np.float32)
    W0 = np.ascontiguousarray(np.asarray(W0, dtype=np.float32))
    W1 = np.ascontiguousarray(np.asarray(W1, dtype=np.float32))
    W2 = np.asarray(W2, dtype=np.float32)
    b0 = np.asarray(b0, dtype=np.float32)
    b1 = np.asarray(b1, dtype=np.float32)
    b2 = np.asarray(b2, dtype=np.float32)

    nc = _build()

    w1h = W1.astype(np.float16)
    w2h = np.ascontiguousarray(W2.astype(np.float16).reshape(KH, 128).T)
    b0r = np.ascontiguousarray(b0.reshape(MH, 128).T)
    b1r = np.ascontiguousarray(b1.reshape(MH, 128).T)
    b2r = b2.reshape(1, 1)

    in_maps = []
    for c in range(NCORES):
        shard = x[c * SHARD:(c + 1) * SHARD]
        in_maps.append(
            {
                "xt": np.ascontiguousarray(shard.T.astype(np.float16)),
                "w0": W0.astype(np.float16),
                "w1": w1h,
                "w2": w2h,
                "b0": b0r,
                "b1": b1r,
                "b2": b2r,
            }
        )

    # The first execution of a freshly-compiled NEFF intermittently hits a
    # transient device error (NRT_EXEC_UNIT_UNRECOVERABLE); a retry succeeds.
    import time as _time

    last_err = None
    for _attempt in range(3):
        try:
            res = run_bass_kernel_spmd(nc, in_maps, core_ids=list(range(NCORES)))
            break
        except Exception as e:  # noqa: BLE001 - retry transient device faults
            last_err = e
            _time.sleep(3.0)
    else:
        raise last_err
    LAST_RESULTS = res

    out = np.concatenate([res.results[c]["out"][0] for c in range(NCORES)])
    alpha = np.concatenate([res.results[c]["alpha"][0] for c in range(NCORES)])
    return out[:, None].astype(np.float32), alpha[:, None].astype(np.float32)



# revision 5
# speedup vs baseline: 1.0873x; 1.0873x over previous
"""AdapLSNet MLP kernel for 8 TRN2 NeuronCores (data-parallel).

reference:
    h  = elu(x @ W0 + b0)
    h  = elu(h @ W1 + b1)
    out = sigmoid(h @ W2 + b2)          # [B, 1]
    alpha = piecewise(out)               # a=0.1, b=0.2, c=0.8
    returns (out, alpha)

Strategy
- Shard batch (32768) across 8 cores (4096 rows each); replicate weights.
- Host pre-transposes each x shard to x^T [1024, 4096] so every layer's
  activations live in [feature(partitions), batch(free)] layout; weights
  W0/W1 are already in [K, M] layout for the stationary operand.  No
  on-device transposes.
- L1/L2 in fp16 (full PE rate, FWL weight loads, half the DMA/SBUF
  bytes; measured end-to-end rel err 1.3e-3 vs the 2e-2 gate).  fp8
  DoubleRow was evaluated and rejected: alpha has only ~8 nonzero tail
  entries, and fp8 noise on `out` gives alpha rel err 0.12-0.24.
- Single fused pass: W0 (fp16) and W1 (fp16) are SBUF-resident, so h1
  never leaves the chip.  Weight slabs stay [128, 2048]; their DMAs are
  emitted STRIP-MAJOR (strip s of every slab before strip s+1) so
  arrival order matches the m-tile consumption order, split across the
  sync (HWDGE) and gpsimd (SWDGE) queue families for bandwidth.
- DMA order: b0/b1/b2/w2 FIRST (b0 gates every L1 ScalarE activation;
  emitting it last serialized the whole startup behind 12MB of weights
  and caused a 23us PE stall + a 48us HAM half-clock window), then xt0,
  W0, xt1, xt2, W1.
- Software pipeline: L1 runs three batch-chunks ahead of L2 so the PE
  has L1 work while W1 streams in at startup.
- L3 (h2 @ W2, M=1) runs OFF the PE: per m-tile a single DVE
  scalar_tensor_tensor accumulates acc += w2[:,m] (x) h2 (per-partition
  scalar multiply), and one f32r ones-vector matmul per chunk reduces
  the 128 partials -> z3 [1,512].  This frees 16 full 512-col MM slots
  per chunk (~26us of PE time total) vs doing L3 as packed matmuls.
- elu(z) = min(exp(z) - 1, relu(z)): 2 ScalarE LUT ops reading PSUM with
  the bias fused + 1 fused VectorE (e-1) min r op.
- alpha = relu(-0.5*out + 0.1) + relu(0.5*out - 0.4)  (exact identity for
  out in [0,1]).
- PE warmup matmuls keep the HAM clock gate released (2.4 GHz) across
  the initial DMA wait.
"""

import numpy as np

BATCH = 32768
DIN = 1024
DH = 2048
NCORES = 8
SHARD = BATCH // NCORES          # 4096
CHUNK = 512
NCH = SHARD // CHUNK             # 8
KI = DIN // 128                  # 8
KH = DH // 128                   # 16
MH = DH // 128                   # 16
NH1S = 48                        # h1 slots (fp16 [128,512], 1KB each; 3 chunks)
NXTS = 14                        # xt slots (fp16 [128,512], 1KB each)
NWARM = 48                       # PE warmup matmuls (HAM un-throttle)


def _install_profile_shim():
    """Allow trace=True under axon (exec_time_ns capture) if possible."""
    import sys
    import types

    try:
        import antenv

        if "antenv.axon_hooks" in sys.modules:
            return
        mod = types.ModuleType("antenv.axon_hooks")
        _hook = [None]
        mod.set_axon_ntff_profile_hook = lambda h: _hook.__setitem__(0, h)
        mod.get_axon_ntff_profile_hook = lambda: _hook[0]
        sys.modules["antenv.axon_hooks"] = mod
        antenv.axon_hooks = mod
        try:
            from trn_agent_boot.trn_boot import _ntff_profile_via_ctypes

            mod.set_axon_ntff_profile_hook(
                _ntff_profile_via_ctypes("/opt/axon/libaxon_pjrt.so")
            )
        except Exception:
            pass
    except Exception:
        pass


_NC_CACHE = None


def _build():
    global _NC_CACHE
    if _NC_CACHE is not None:
        return _NC_CACHE

    import concourse.mybir as mybir
    import concourse.tile as tile
    from concourse import bacc

    F32R = mybir.dt.float32r
    F32 = mybir.dt.float32
    F16 = mybir.dt.float16
    AF = mybir.ActivationFunctionType
    ALU = mybir.AluOpType

    nc = bacc.Bacc("TRN2", target_bir_lowering=False)

    xt_ext = nc.declare_dram_parameter("xt", [DIN, SHARD], F16, isOutput=False)
    w0_ext = nc.declare_dram_parameter("w0", [DIN, DH], F16, isOutput=False)
    w1_ext = nc.declare_dram_parameter("w1", [DH, DH], F16, isOutput=False)
    w2_ext = nc.declare_dram_parameter("w2", [128, KH], F32, isOutput=False)
    b0_ext = nc.declare_dram_parameter("b0", [128, MH], F32, isOutput=False)
    b1_ext = nc.declare_dram_parameter("b1", [128, MH], F32, isOutput=False)
    b2_ext = nc.declare_dram_parameter("b2", [1, 1], F32, isOutput=False)
    out_ext = nc.declare_dram_parameter("out", [1, SHARD], F32, isOutput=True)
    alpha_ext = nc.declare_dram_parameter("alpha", [1, SHARD], F32, isOutput=True)

    with tile.TileContext(nc) as tc:
        with (
            tc.tile_pool(name="w0p", bufs=1) as w0p,
            tc.tile_pool(name="w1p", bufs=1) as w1p,
            tc.tile_pool(name="xtp", bufs=1) as xtp,
            tc.tile_pool(name="h1p", bufs=1) as h1p,
            tc.tile_pool(name="hpool", bufs=2) as hpool,
            tc.tile_pool(name="h2p", bufs=4) as h2p,
            tc.tile_pool(name="accp", bufs=1) as accp,
            tc.tile_pool(name="redp", bufs=2) as redp,
            tc.tile_pool(name="cst", bufs=1) as cst,
            tc.tile_pool(name="ps", bufs=6, space="PSUM") as ps,
            tc.tile_pool(name="ops", bufs=2, space="PSUM") as ops,
        ):
            w0_sb = [
                w0p.tile([128, DH], F16, tag=f"w0_{k}", name=f"w0_{k}")
                for k in range(KI)
            ]
            w1_sb = [
                w1p.tile([128, DH], F16, tag=f"w1_{k}", name=f"w1_{k}")
                for k in range(KH)
            ]

            def w0_lhsT(k, m):
                return w0_sb[k][:, m * 128:(m + 1) * 128]

            def w1_lhsT(k, m):
                return w1_sb[k][:, m * 128:(m + 1) * 128]

            def emit_xt(n, halves=False):
                tiles = []
                base = (KI * n) % NXTS
                for k in range(KI):
                    t = xtp.tile(
                        [128, CHUNK], F16, tag=f"xt{(base + k) % NXTS}",
                        name=f"xt_{n}_{k}",
                    )
                    src = xt_ext[k * 128:(k + 1) * 128,
                                 n * CHUNK:(n + 1) * CHUNK]
                    if halves:
                        nc.sync.dma_start(t[:, 0:256], src[:, 0:256])
                        nc.sync.dma_start(t[:, 256:512], src[:, 256:512])
                    else:
                        nc.sync.dma_start(t[:], src)
                    tiles.append(t)
                return tiles

            # --- small, first-use-critical tensors FIRST: b0 gates every
            # L1 ScalarE activation (and thence psum recycling) ---
            b0_sb = cst.tile([128, MH], F32, tag="b0", name="b0")
            nc.sync.dma_start(b0_sb[:], b0_ext[:])
            b1_sb = cst.tile([128, MH], F32, tag="b1", name="b1")
            nc.sync.dma_start(b1_sb[:], b1_ext[:])
            b2_sb = cst.tile([1, 1], F32, tag="b2", name="b2")
            nc.sync.dma_start(b2_sb[:], b2_ext[:])
            w2_sb = cst.tile([128, KH], F32, tag="w2", name="w2")
            nc.sync.dma_start(w2_sb[:], w2_ext[:])
            c_b1 = cst.tile([1, 1], F32, tag="c_b1", name="c_b1")
            c_b2 = cst.tile([1, 1], F32, tag="c_b2", name="c_b2")
            c_sn = cst.tile([1, 1], F32, tag="c_sn", name="c_sn")
            c_sp = cst.tile([1, 1], F32, tag="c_sp", name="c_sp")
            nc.vector.memset(c_b1[:], 0.1)
            nc.vector.memset(c_b2[:], -0.4)
            nc.vector.memset(c_sn[:], -0.5)
            nc.vector.memset(c_sp[:], 0.5)
            ones_sb = cst.tile([128, 1], F16, tag="ones", name="ones")
            nc.vector.memset(ones_sb[:], 1.0)

            # --- xt0 next (first-chunk moving operand) ---
            xt_tiles = {0: emit_xt(0, halves=True)}

            # --- W0 STRIP-MAJOR: strip s of every slab before strip s+1,
            # matching the L1 m-tile consumption order (m-pair 2s,2s+1
            # reads cols [256s, 256s+256) of all 8 slabs).  Alternate
            # HWDGE/SWDGE queue families per transfer. ---
            W0S = DH // 8            # 256-col strips
            for s in range(8):
                for k in range(KI):
                    eng = nc.sync if ((s + k) % 2 == 0) else nc.gpsimd
                    eng.dma_start(
                        w0_sb[k][:, s * W0S:(s + 1) * W0S],
                        w0_ext[k * 128:(k + 1) * 128, s * W0S:(s + 1) * W0S],
                    )

            xt_tiles[1] = emit_xt(1, halves=True)
            xt_tiles[2] = emit_xt(2, halves=True)

            # --- W1 strip-major (4 strips of 512 cols) ---
            W1S = DH // 4
            for s in range(4):
                for k in range(KH):
                    eng = nc.sync if ((s + k) % 2 == 0) else nc.gpsimd
                    eng.dma_start(
                        w1_sb[k][:, s * W1S:(s + 1) * W1S],
                        w1_ext[k * 128:(k + 1) * 128, s * W1S:(s + 1) * W1S],
                    )

            # PE warmup: dependency-free matmuls on a memset tile keep the
            # PE busy during the initial DMA wait so the HAM clock gate is
            # already released (2.4 GHz) when real matmuls start.
            wu = hpool.tile([128, CHUNK], F16, tag="e", name="wu")
            nc.vector.memset(wu[:], 0.0)
            for i in range(NWARM):
                wps = ps.tile([128, CHUNK], F32, tag="ps", name=f"wups_{i}")
                nc.tensor.matmul(
                    wps[:], wu[:, 0:128], wu[:], start=True, stop=True,
                )

            h1_tiles = {}

            def l1_chunk(n):
                """L1: h1(n) = elu(W0.T @ xT(n) + b0), kept in SBUF."""
                xt_sb = xt_tiles.pop(n)
                h1base = (MH * n) % NH1S
                tiles = []
                for m in range(MH):
                    psum = ps.tile([128, CHUNK], F32, tag="ps",
                                   name=f"psA_{n}_{m}")
                    for k in range(KI):
                        nc.tensor.matmul(
                            psum[:], w0_lhsT(k, m), xt_sb[k][:],
                            start=(k == 0), stop=(k == KI - 1),
                        )
                    e = hpool.tile([128, CHUNK], F32, tag="e", name="e")
                    r = hpool.tile([128, CHUNK], F32, tag="r", name="r")
                    nc.scalar.activation(e[:], psum[:], AF.Exp,
                                         bias=b0_sb[:, m:m + 1])
                    nc.scalar.activation(r[:], psum[:], AF.Relu,
                                         bias=b0_sb[:, m:m + 1])
                    h1 = h1p.tile(
                        [128, CHUNK], F16, tag=f"h{(h1base + m) % NH1S}",
                        name=f"h1_{n}_{m}",
                    )
                    nc.vector.scalar_tensor_tensor(
                        h1[:], e[:], 1.0, r[:], ALU.subtract, ALU.min
                    )
                    tiles.append(h1)
                h1_tiles[n] = tiles

            def l2_chunk(n):
                """L2 + L3 + sigmoid + alpha for chunk n.

                L3 runs off the PE: a DVE scalar_tensor_tensor chain
                accumulates acc += w2[:,m] (x) h2 per m-tile, then one
                f32r ones-vector matmul reduces partitions -> z3 [1,512].
                """
                h1_sb = h1_tiles.pop(n)
                prev = None
                for m in range(MH):
                    psum = ps.tile([128, CHUNK], F32, tag="ps",
                                   name=f"psB_{n}_{m}")
                    for k in range(KH):
                        nc.tensor.matmul(
                            psum[:], w1_lhsT(k, m), h1_sb[k][:],
                            start=(k == 0), stop=(k == KH - 1),
                        )
                    e = hpool.tile([128, CHUNK], F32, tag="e", name="e")
                    r = hpool.tile([128, CHUNK], F32, tag="r", name="r")
                    nc.scalar.activation(e[:], psum[:], AF.Exp,
                                         bias=b1_sb[:, m:m + 1])
                    nc.scalar.activation(r[:], psum[:], AF.Relu,
                                         bias=b1_sb[:, m:m + 1])
                    h2 = h2p.tile([128, CHUNK], F16, tag="h2", name="h2")
                    nc.vector.scalar_tensor_tensor(
                        h2[:], e[:], 1.0, r[:], ALU.subtract, ALU.min
                    )
                    a = accp.tile(
                        [128, CHUNK], F16 if m == MH - 1 else F32,
                        tag=f"acc{(2 * n + (m % 2)) % 4}",
                        name=f"acc_{n}_{m}",
                    )
                    if m == 0:
                        nc.vector.tensor_scalar(
                            a[:], h2[:], w2_sb[:, 0:1], None, ALU.mult,
                        )
                    else:
                        nc.vector.scalar_tensor_tensor(
                            a[:], h2[:], w2_sb[:, m:m + 1], prev[:],
                            ALU.mult, ALU.add,
                        )
                    prev = a
                out_ps = ops.tile([1, CHUNK], F32, tag="ops",
                                  name=f"outps_{n}")
                nc.tensor.matmul(
                    out_ps[:], ones_sb[:], prev[:], start=True, stop=True,
                )
                o = hpool.tile([1, CHUNK], F32, tag="e", name="o")
                nc.scalar.activation(o[:], out_ps[:], AF.Sigmoid,
                                     bias=b2_sb[:])
                r1 = hpool.tile([1, CHUNK], F32, tag="r", name="r1")
                r2 = redp.tile([1, CHUNK], F32, tag="tred", name="r2")
                nc.scalar.activation(r1[:], o[:], AF.Relu,
                                     bias=c_b1[:], scale=c_sn[:])
                nc.scalar.activation(r2[:], o[:], AF.Relu,
                                     bias=c_b2[:], scale=c_sp[:])
                al = hpool.tile([1, CHUNK], F32, tag="e", name="al")
                nc.vector.tensor_tensor(al[:], r1[:], r2[:], ALU.add)
                nc.sync.dma_start(out_ext[0:1, n * CHUNK:(n + 1) * CHUNK], o[:])
                nc.sync.dma_start(
                    alpha_ext[0:1, n * CHUNK:(n + 1) * CHUNK], al[:]
                )

            # pipeline: L1 three chunks ahead of L2 (consume chunk n-3
            # BEFORE L1(n) writes into its ring slots - else deadlock)
            l1_chunk(0)
            l1_chunk(1)
            xt_tiles[3] = emit_xt(3)
            l1_chunk(2)
            for n in range(3, NCH):
                l2_chunk(n - 3)
                l1_chunk(n)
                if n + 1 < NCH:
                    xt_tiles[n + 1] = emit_xt(n + 1)
            for n in range(NCH - 3, NCH):
                l2_chunk(n)

    nc.compile()
    _NC_CACHE = nc
    return nc


LAST_RESULTS = None


def kernel(x, W0, b0, W1, b1, W2, b2):
    global LAST_RESULTS
    _install_profile_shim()
    from concourse.bass_utils import run_bass_kernel_spmd

    x = np.asarray(x, dtype=np.float32)
    W0 = np.ascontiguousarray(np.asarray(W0, dtype=np.float32))
    W1 = np.ascontiguousarray(np.asarray(W1, dtype=np.float32))
    W2 = np.asarray(W2, dtype=np.float32)
    b0 = np.asarray(b0, dtype=np.float32)
    b1 = np.asarray(b1, dtype=np.float32)
    b2 = np.asarray(b2, dtype=np.float32)

    nc = _build()

    w1h = W1.astype(np.float16)
    w2r = np.ascontiguousarray(W2.reshape(KH, 128).T.astype(np.float32))
    b0r = np.ascontiguousarray(b0.reshape(MH, 128).T)
    b1r = np.ascontiguousarray(b1.reshape(MH, 128).T)
    b2r = b2.reshape(1, 1)

    in_maps = []
    for c in range(NCORES):
        shard = x[c * SHARD:(c + 1) * SHARD]
        in_maps.append(
            {
                "xt": np.ascontiguousarray(shard.T.astype(np.float16)),
                "w0": W0.astype(np.float16),
                "w1": w1h,
                "w2": w2r,
                "b0": b0r,
                "b1": b1r,
                "b2": b2r,
            }
        )

    # The first execution of a freshly-compiled NEFF intermittently hits a
    # transient device error (NRT_EXEC_UNIT_UNRECOVERABLE); a retry succeeds.
    import time as _time

    last_err = None
    for _attempt in range(3):
        try:
            res = run_bass_kernel_spmd(nc, in_maps, core_ids=list(range(NCORES)))
            break
        except Exception as e:  # noqa: BLE001 - retry transient device faults
            last_err = e
            _time.sleep(3.0)
    else:
        raise last_err
    LAST_RESULTS = res

    out = np.concatenate([res.results[c]["out"][0] for c in range(NCORES)])
    alpha = np.concatenate([res.results[c]["alpha"][0] for c in range(NCORES)])
    return out[:, None].astype(np.float32), alpha[:, None].astype(np.float32)
